# revision 1
# baseline (speedup 1.0000x reference)
"""Multi-head attention (B=8, P=1024, D=768, H=12) on 8 TRN2 NeuronCores.

Strategy: pure data parallelism — batch element b runs on core b (no
collectives). Host pre-transposes x and casts operands to bf16; each core
computes QK^T/softmax/AV/proj for its batch element with all matmuls on the
TensorEngine (bf16, fp32 PSUM accumulation), exp on the ScalarEngine, and
evacuations/normalization on the VectorEngine.

Self-contained: builds + compiles the Bass kernel on first call, runs via
PJRT (axon) across cores 0-7, and reassembles full outputs. Returns the
tuple (out, weighted_avg), matching the reference.
"""

import numpy as np
from contextlib import ExitStack

import jax
import jax.numpy as jnp
from jax.experimental.shard_map import shard_map
from jax.sharding import Mesh, PartitionSpec

import bass_rust
import concourse.bass as bass
import concourse.tile as tile
from concourse import mybir
from concourse import bass2jax

B, P, D = 8, 1024, 768
H = 12
HD = D // H            # 64
SCALE = HD ** -0.5     # 0.125
N_CORES = 8
KT = D // 128          # 6 contraction tiles over d
QT = P // 128          # 8 tiles over sequence
BF = mybir.dt.bfloat16
F32 = mybir.dt.float32
NP_BF16 = np.dtype(mybir.dt.np(BF))

IN_NAMES = ["xT", "wqk", "wv", "wph", "bqk", "bv", "bp", "ident", "selmat"]
OUT_NAMES = ["out", "wa"]


def _split_excess_waits(nc, max_waits=1):
    """This container's walrus build rejects instructions carrying more than
    one sync wait. Hoist excess waits onto same-engine no-ops inserted just
    before the overloaded instruction (engine queues execute in order, so
    wait-for-all-before-exec semantics are preserved)."""
    ctr = 0
    for bb in nc.main_func.blocks:
        newlist = []
        dirty = False
        for inst in bb.instructions:
            si = inst.sync_info
            waits = list(si.on_wait) if (si is not None and si.on_wait) else []
            if len(waits) > max_waits:
                excess, keep = waits[:-max_waits], waits[-max_waits:]
                for i in range(0, len(excess), max_waits):
                    chunk = excess[i : i + max_waits]
                    nop = bass_rust.InstNoOp(name=f"WSPILL-{ctr}")
                    ctr += 1
                    nop.engine = inst.engine
                    nop.sync_info = bass_rust.SyncInfo(on_wait=chunk, on_update=[])
                    newlist.append(nop)
                inst.sync_info = bass_rust.SyncInfo(
                    on_wait=keep, on_update=list(si.on_update or [])
                )
                dirty = True
            newlist.append(inst)
        if dirty:
            bb.instructions = newlist
    return ctr


def _bcast_ap(dram_ap, parts):
    """Partition-stride-0 DMA source view of a 1-D DRAM tensor: [n] -> [parts, n]."""
    return bass.AP(
        tensor=dram_ap.tensor,
        offset=dram_ap.offset,
        ap=[[0, parts]] + list(dram_ap.ap),
    )


def build_nc(split_waits=True, max_phase=6, loop_n=None, unroll=1, probe=None):
    nc = bass.Bass(target_bir_lowering=False)

    xT_e = nc.declare_dram_parameter("xT", [D, P], BF, isOutput=False)
    wqk_e = nc.declare_dram_parameter("wqk", [D, 2 * D], BF, isOutput=False)
    wv_e = nc.declare_dram_parameter("wv", [D, D], BF, isOutput=False)
    wph_e = nc.declare_dram_parameter("wph", [H // 2, 128, D], BF, isOutput=False)
    bqk_e = nc.declare_dram_parameter("bqk", [128, 2 * D // 128], F32, isOutput=False)
    bv_e = nc.declare_dram_parameter("bv", [D], F32, isOutput=False)
    bp_e = nc.declare_dram_parameter("bp", [D], F32, isOutput=False)
    id_e = nc.declare_dram_parameter("ident", [128, 128], BF, isOutput=False)
    sel_e = nc.declare_dram_parameter("selmat", [H, H * HD], BF, isOutput=False)
    out_e = nc.declare_dram_parameter("out", [P, D], BF, isOutput=True)
    wa_e = nc.declare_dram_parameter("wa", [P, D], BF, isOutput=True)

    EXP = mybir.ActivationFunctionType.Exp
    LN = mybir.ActivationFunctionType.Ln

    with tile.TileContext(nc) as tc, ExitStack() as ctx:
        if loop_n is not None:
            ctx.enter_context(tc.For_i(0, loop_n, 1))
        const = ctx.enter_context(tc.tile_pool(name="const", bufs=1))
        qkp = ctx.enter_context(tc.tile_pool(name="qkp", bufs=1))
        vxp = ctx.enter_context(tc.tile_pool(name="vxp", bufs=1))
        wtp = ctx.enter_context(tc.tile_pool(name="wtp", bufs=1))
        ptp = ctx.enter_context(tc.tile_pool(name="ptp", bufs=6))
        stgp = ctx.enter_context(tc.tile_pool(name="stgp", bufs=1))
        outp = ctx.enter_context(tc.tile_pool(name="outp", bufs=4))
        psum = ctx.enter_context(tc.tile_pool(name="psum", bufs=4, space="PSUM"))

        for _it in range(unroll):
            # ---- constant loads --------------------------------------------
            xT = [const.tile([128, P], BF, tag=f"xT{k}", name=f"xT{k}") for k in range(KT)]
            wqk = [const.tile([128, 2 * D], BF, tag=f"wqk{k}", name=f"wqk{k}") for k in range(KT)]
            wv = [const.tile([128, D], BF, tag=f"wv{k}", name=f"wv{k}") for k in range(KT)]
            wp = [const.tile([128, D], BF, tag=f"wp{p}", name=f"wp{p}") for p in range(H // 2)]
            bqk = const.tile([128, 2 * D // 128], F32, tag="bqk", name="bqk")
            bvb = const.tile([128, D], F32, tag="bvb", name="bvb")
            bpb = const.tile([128, D], F32, tag="bpb", name="bpb")
            ident = const.tile([128, 128], BF, tag="ident", name="ident")
            selmat = const.tile([H, H * HD], BF, tag="selmat", name="selmat")

            # DMA order = first-use order: x/qk weights and v weights lead;
            # proj/ident/selmat are tail-only and queue last.
            for k in range(KT):
                nc.sync.dma_start(out=xT[k], in_=xT_e[k * 128 : (k + 1) * 128, :])
                nc.sync.dma_start(out=wqk[k], in_=wqk_e[k * 128 : (k + 1) * 128, :])
                nc.gpsimd.dma_start(out=wv[k], in_=wv_e[k * 128 : (k + 1) * 128, :])
            nc.gpsimd.dma_start(out=bqk, in_=bqk_e[:])
            nc.gpsimd.dma_start(out=bvb, in_=_bcast_ap(bv_e[:], 128))
            nc.gpsimd.dma_start(out=selmat, in_=sel_e[:])
            for p in range(H // 2):
                nc.gpsimd.dma_start(out=wp[p], in_=wph_e[p])
            nc.gpsimd.dma_start(out=bpb, in_=_bcast_ap(bp_e[:], 128))
            nc.gpsimd.dma_start(out=ident, in_=id_e[:])

            if max_phase < 2:
                for qt in range(QT):
                    for c in range(D // 128):
                        nc.sync.dma_start(out=out_e[qt * 128 : (qt + 1) * 128, c * 128 : (c + 1) * 128], in_=ident)
                        nc.gpsimd.dma_start(out=wa_e[qt * 128 : (qt + 1) * 128, c * 128 : (c + 1) * 128], in_=ident)
                continue

            # ---- phase 1: qT / kT = (w_qk)^T @ x^T  [feature-major] --------
            # qkT[m] rows = features m*128..; m 0..5 -> q, 6..11 -> k.
            # Emission is per head-pair: pair pr needs tiles {pr, 6+pr} only,
            # so remaining pairs' qkT matmuls are inserted between attention
            # pairs (they fill the PE while the ScalarEngine runs exp).
            qkT = [qkp.tile([128, P], BF, tag=f"qkT{m}", name=f"qkT{m}") for m in range(2 * D // 128)]

            def emit_qkT(ms):
                for m in ms:
                    for j in range(2):
                        ps = psum.tile([128, 1024], F32, tag="ps", name="ps", padded_shape=None)[:, 0:512]
                        for k in range(KT):
                            nc.tensor.matmul(
                                ps,
                                lhsT=wqk[k][:, m * 128 : (m + 1) * 128],
                                rhs=xT[k][:, j * 512 : (j + 1) * 512],
                                start=(k == 0),
                                stop=(k == KT - 1),
                            )
                        nc.vector.tensor_scalar_add(
                            qkT[m][:, j * 512 : (j + 1) * 512], ps, bqk[:, m : m + 1]
                        )

            first_ms = [0, 6] if (probe is None and max_phase >= 3) else ([] if probe == 'v' else list(range(12)))
            emit_qkT(first_ms)

            # ---- phase 2: v natural [seq-major] with ones column ------------
            # vext[p][:, h, 0:64] = v_h rows p*128..; vext[p][:, h, 64] = 1.0
            vext = [vxp.tile([128, H, HD + 1], BF, tag=f"vext{p}", name=f"vext{p}") for p in range(QT)]
            for p in range(QT if probe != 'qkT' else 0):
                nc.vector.memset(vext[p][:, :, HD : HD + 1], 1.0)
                for (c0, cw) in ((0, 512), (512, 256)):
                    ps = psum.tile([128, 1024], F32, tag="ps", name="ps", padded_shape=None)[:, 0:512]
                    for k in range(KT):
                        nc.tensor.matmul(
                            ps[:, :cw],
                            lhsT=xT[k][:, p * 128 : (p + 1) * 128],
                            rhs=wv[k][:, c0 : c0 + cw],
                            start=(k == 0),
                            stop=(k == KT - 1),
                        )
                    nh = cw // HD
                    nc.vector.tensor_add(
                        vext[p][:, c0 // HD : c0 // HD + nh, 0:HD],
                        ps[:, :cw].rearrange("p (h d) -> p h d", d=HD),
                        bvb[:, c0 : c0 + cw].rearrange("p (h d) -> p h d", d=HD),
                    )

            # ---- phase 3: attention per head pair ---------------------------
            # S^T tiles: [k-positions, q] via lhsT=kT slice (stationary),
            # rhs=qT (moving). Even head at partitions 0-63, odd at 64-127 ->
            # concurrent row-tiled matmuls. exp on ScalarE (scale folded).
            # AV: lhsT=[v_h | ones] so psum row 64 = softmax denominator.
            # AV matmuls lag the S/exp pipeline by one k-tile so the PE never
            # waits on the ScalarEngine.
            waTp = [wtp.tile([128, P], BF, tag=f"waTp{p}", name=f"waTp{p}") for p in range(H // 2)]
            dens12 = stgp.tile([H, P], BF, tag="dens12", name="dens12")
            recip12 = stgp.tile([H, P], F32, tag="recip12", name="recip12")
            recip12b = stgp.tile([H, P], BF, tag="recip12b", name="recip12b")
            nc.vector.memset(dens12, 1.0)
            stg_tiles = {}

            def emit_norm(head_range):
                # normalize: broadcast recip row h to 64 partitions via a
                # selector matmul (psr[d, q] = recip[h, q]); multiply the
                # staged AV rows; merge odd heads into the pair tile's upper
                # partitions by DMA
                for h in head_range:
                    psr = psum.tile([HD, P], F32, tag="ps", name="ps")
                    for j in range(2):
                        nc.tensor.matmul(
                            psr[:, j * 512 : (j + 1) * 512],
                            lhsT=selmat[:, h * HD : (h + 1) * HD],
                            rhs=recip12b[0:H, j * 512 : (j + 1) * 512],
                            start=True,
                            stop=True,
                        )
                    if h % 2 == 0:
                        nc.vector.tensor_mul(
                            waTp[h // 2][0:HD, :], stg_tiles[h][0:HD, :], psr
                        )
                    else:
                        wt = outp.tile([HD, P], BF, tag="wtmp", name="wtmp")
                        nc.vector.tensor_mul(wt, stg_tiles[h][0:HD, :], psr)
                        nc.sync.dma_start(out=waTp[h // 2][HD:128, :], in_=wt)

            def emit_wa(qt):
                # all six pair-transposes into one 2-bank psum, one evac,
                # one row-contiguous DMA
                psw = psum.tile([128, D], F32, tag="ps", name="ps")
                for p in range(H // 2):
                    nc.tensor.matmul(
                        psw[:, p * 128 : (p + 1) * 128],
                        lhsT=waTp[p][:, qt * 128 : (qt + 1) * 128],
                        rhs=ident,
                        start=True,
                        stop=True,
                    )
                wa_sb = outp.tile([128, D], BF, tag="wa_sb", name="wa_sb")
                nc.scalar.copy(wa_sb, psw)
                nc.gpsimd.dma_start(out=wa_e[qt * 128 : (qt + 1) * 128, :], in_=wa_sb)

            for pr in range(H // 2 if max_phase >= 3 else 0):
                heads = (2 * pr, 2 * pr + 1)
                psav = {h: psum.tile([HD + 1, P], F32, tag="ps", name="ps") for h in heads}
                pt_prev = None
                for kt in range(QT + 1):
                    if kt > 0:  # AV for previous k-tile (exp already done)
                        for h in heads:
                            for j in range(2):
                                nc.tensor.matmul(
                                    psav[h][:, j * 512 : (j + 1) * 512],
                                    lhsT=vext[kt - 1][:, h, :],
                                    rhs=pt_prev[h][:, j * 512 : (j + 1) * 512],
                                    start=(kt - 1 == 0),
                                    stop=(kt - 1 == QT - 1),
                                )
                    if kt < QT:
                        pss = {h: psum.tile([128, P], F32, tag="ps", name="ps") for h in heads}
                        for j in range(2):
                            for h in heads:
                                base = (h % 2) * 64
                                nc.tensor.matmul(
                                    pss[h][:, j * 512 : (j + 1) * 512],
                                    lhsT=qkT[6 + h // 2][base : base + 64, kt * 128 : (kt + 1) * 128],
                                    rhs=qkT[h // 2][base : base + 64, j * 512 : (j + 1) * 512],
                                    start=True,
                                    stop=True,
                                )
                        cur = {}
                        for h in heads:
                            pt = ptp.tile([128, P], BF, tag="pt")
                            nc.scalar.activation(pt, pss[h], EXP, scale=SCALE)
                            cur[h] = pt
                        pt_prev = cur
                for h in heads:
                    stg = stgp.tile([HD + 1, P], BF, tag=f"stg{h}", name=f"stg{h}")
                    nc.vector.tensor_copy(stg, psav[h])
                    nc.gpsimd.dma_start(out=dens12[h : h + 1, :], in_=stg[HD : HD + 1, :])
                    stg_tiles[h] = stg
                if pr + 1 < H // 2:
                    emit_qkT([pr + 1, 6 + pr + 1])
                if max_phase >= 4 and pr in (3, 5):
                    # reciprocal of denominator rows as 1/d = exp(-ln(d)) on
                    # the ScalarEngine (the natural_log set contains both)
                    nc.scalar.activation(recip12[0:12, :], dens12[0:12, :], LN)
                    nc.scalar.activation(recip12b[0:12, :], recip12[0:12, :], EXP, scale=-1.0)

            # ---- phase 4: tail part — late heads' normalize + remaining wa --
            if max_phase >= 4:
                emit_norm(range(0, H))
            if max_phase == 5:
                for qt in range(QT):
                    emit_wa(qt)

            # ---- phase 5: per q-tile, wa transposes fused with proj --------
            for qt in range(QT if max_phase >= 6 else 0):
                emit_wa(qt)
                out_sb = outp.tile([128, D], BF, tag="out_sb", name="out_sb")
                for (c0, cw) in ((0, 512), (512, 256)):
                    ps = psum.tile([128, 1024], F32, tag="ps", name="ps", padded_shape=None)[:, 0:512]
                    for p in range(H // 2):
                        nc.tensor.matmul(
                            ps[:, :cw],
                            lhsT=waTp[p][:, qt * 128 : (qt + 1) * 128],
                            rhs=wp[p][:, c0 : c0 + cw],
                            start=(p == 0),
                            stop=(p == H // 2 - 1),
                        )
                    nc.vector.tensor_add(
                        out_sb[:, c0 : c0 + cw], ps[:, :cw], bpb[:, c0 : c0 + cw]
                    )
                nc.sync.dma_start(out=out_e[qt * 128 : (qt + 1) * 128, :], in_=out_sb)

            if max_phase < 6:
                for qt in range(QT):
                    nc.sync.dma_start(out=out_e[qt * 128 : (qt + 1) * 128, :], in_=qkT[0][:, 0:D])
            if max_phase < 5:
                for qt in range(QT):
                    nc.sync.dma_start(out=wa_e[qt * 128 : (qt + 1) * 128, :], in_=qkT[1][:, 0:D])

    if split_waits:
        _split_excess_waits(nc)
    return nc


def make_in_maps(x, w_qkv, b_qkv, w_proj, b_proj):
    """Host-side shard prep: batch element b -> core b; weights replicated."""
    xf = np.asarray(x, dtype=np.float32)
    wqkv = np.asarray(w_qkv, dtype=np.float32)
    bqkv = np.asarray(b_qkv, dtype=np.float32)
    wproj = np.asarray(w_proj, dtype=np.float32)
    bproj = np.asarray(b_proj, dtype=np.float32)

    wqk = np.ascontiguousarray(wqkv[:, : 2 * D]).astype(NP_BF16)
    wv = np.ascontiguousarray(wqkv[:, 2 * D :]).astype(NP_BF16)
    wph = np.ascontiguousarray(wproj.reshape(H // 2, 128, D)).astype(NP_BF16)
    bqk = np.ascontiguousarray(bqkv[: 2 * D].reshape(2 * D // 128, 128).T)
    bv = np.ascontiguousarray(bqkv[2 * D :])
    ident = np.eye(128, dtype=np.float32).astype(NP_BF16)
    selmat = np.kron(np.eye(H, dtype=np.float32), np.ones((1, HD), np.float32)).astype(NP_BF16)

    in_maps = []
    for b in range(N_CORES):
        in_maps.append(
            {
                "xT": np.ascontiguousarray(xf[b].T).astype(NP_BF16),
                "wqk": wqk,
                "wv": wv,
                "wph": wph,
                "bqk": bqk,
                "bv": bv,
                "bp": bproj,
                "ident": ident,
                "selmat": selmat,
            }
        )
    return in_maps


_CACHE = {}


def _get_nc():
    if "nc" not in _CACHE:
        _CACHE["nc"] = build_nc()
    return _CACHE["nc"]


def run_once(in_maps, nc=None):
    """One 8-core execution via the PJRT redirect path (fresh jit per call;
    NEFF comes from the neuron compile cache after the first call)."""
    if nc is None:
        nc = _get_nc()
    return bass2jax.run_bass_via_pjrt(nc, in_maps, n_cores=N_CORES)


def get_cached_runner(nc=None):
    """Cached-jit repeated executor (mirrors run_bass_via_pjrt, including the
    auto-generated partition_id input supplied last via PartitionIdOp)."""
    if nc is None:
        nc = _get_nc()
    key = ("runner", id(nc))
    if key in _CACHE:
        return _CACHE[key]

    import jax
    from jax.experimental.shard_map import shard_map
    from jax.sharding import Mesh, PartitionSpec

    bass2jax.install_neuronx_cc_hook()
    out_avals = tuple(jax.core.ShapedArray((P, D), NP_BF16) for _ in OUT_NAMES)
    partition_name = nc.partition_id_tensor.name if nc.partition_id_tensor else None
    all_in_names = tuple(IN_NAMES) + tuple(OUT_NAMES)
    if partition_name is not None:
        all_in_names = all_in_names + (partition_name,)

    def _body(*args):
        operands = list(args)
        if partition_name is not None:
            operands.append(bass2jax.partition_id_tensor())
        outs = bass2jax._bass_exec_p.bind(
            *operands,
            out_avals=out_avals,
            in_names=all_in_names,
            out_names=tuple(OUT_NAMES),
            lowering_input_output_aliases=(),
            sim_require_finite=True,
            sim_require_nnan=True,
            nc=nc,
        )
        return tuple(outs)

    devices = jax.devices()[:N_CORES]
    mesh = Mesh(np.asarray(devices), ("core",))
    n_params = len(IN_NAMES)
    n_args = n_params + len(OUT_NAMES)
    jitted = jax.jit(
        shard_map(
            _body,
            mesh=mesh,
            in_specs=(PartitionSpec("core"),) * n_args,
            out_specs=(PartitionSpec("core"),) * len(OUT_NAMES),
            check_rep=False,
        ),
        donate_argnums=tuple(range(n_params, n_args)),
        keep_unused=True,
    )

    def run(concat_in):
        zeros = [np.zeros((N_CORES * P, D), NP_BF16) for _ in OUT_NAMES]
        outs = jitted(*concat_in, *zeros)
        jax.block_until_ready(outs)
        return outs

    _CACHE[key] = run
    return run


def kernel(x, w_qkv, b_qkv, w_proj, b_proj):
    in_maps = make_in_maps(x, w_qkv, b_qkv, w_proj, b_proj)
    results = run_once(in_maps)
    out = np.stack([results[b]["out"] for b in range(N_CORES)]).astype(np.float32)
    wa = np.stack([results[b]["wa"] for b in range(N_CORES)]).astype(np.float32)
    return (out, wa)

